# revision 41
# baseline (speedup 1.0000x reference)
"""Trainium2 Bass kernel for nn_DirectMPIGOGate (trilinear MoE-gate).

Strategy (8-core SPMD, data-parallel over points):
  - Host: sort points by x-slab (64 chunks of 2 x-planes), pad each chunk to a
    fixed size, precompute int16 in-chunk voxel ids, the 8 trilinear corner
    weights, and per-point view-dir embeddings (27 dims + ones row).
  - Grid is repacked into a "neighborhood table": one 256B bf16 row per cell
    holding all 8 corners x 12 channels, so each point costs ONE gather.
  - Device per 512-point slice: transposed dma_gather -> GT[96,512] (feature
    major), expand weights [8,512]->[96,512] via a one-hot matmul, elementwise
    multiply, then the 3-layer MLP entirely as K-on-partition matmuls with
    relu+bias fused into ACT PSUM->SBUF evacuations. Logits are written
    feature-major and unpacked on the host.
"""

import os
import numpy as np
import ml_dtypes

bf16 = ml_dtypes.bfloat16

# ---- problem constants (hardcoded per contract) ----
M = 1048576
N_RAYS = 8192
C = 12
GS = 128                 # grid side
PE_F = 4                 # viewbase_pe
WID = 64
E = 8
N_CORES = 8

N_CHUNKS = 64            # x-slabs of 2 planes
CHUNKS_PER_CORE = N_CHUNKS // N_CORES
ROWS_PER_CHUNK = 2 * GS * GS          # 32768 (fits int16)
S_CHUNK = 16896                       # padded points per chunk (mean 16384 + 4 sigma)
TILES = [4096, 4096, 4096, 4096, 512]  # gather-call tiles per chunk
assert sum(TILES) == S_CHUNK
M_CORE = CHUNKS_PER_CORE * S_CHUNK    # 135168
IDXC = S_CHUNK // 16                  # idx16 cols per chunk (1056)

_cache = {}


def _build_program():
    import concourse.bass as bass
    import concourse.bacc as bacc
    import concourse.mybir as mybir
    import concourse.tile as tile

    nc = bacc.Bacc(num_swdge_queues=4, dynamic_dma_scratch_size=65536)
    P = 128
    GQ = 4096                       # idxs per gather call (deep ring fits 5)
    gather_n = [0]                  # global SWDGE op counter (queue = n % 4)

    tbl = nc.dram_tensor("tbl", [CHUNKS_PER_CORE * ROWS_PER_CHUNK, P],
                         mybir.dt.bfloat16, kind="ExternalInput")
    idx16 = nc.dram_tensor("idx16", [P, CHUNKS_PER_CORE * IDXC],
                           mybir.dt.int16, kind="ExternalInput")
    w8g = nc.dram_tensor("w8g", [P, (M_CORE // P) * 8], mybir.dt.bfloat16,
                         kind="ExternalInput")
    vdt = nc.dram_tensor("vdt", [32, M_CORE], mybir.dt.bfloat16, kind="ExternalInput")
    w0f = nc.dram_tensor("w0f", [P, WID], mybir.dt.bfloat16, kind="ExternalInput")
    w1bd = nc.dram_tensor("w1bd", [P, P], mybir.dt.bfloat16, kind="ExternalInput")
    w2bd = nc.dram_tensor("w2bd", [P, 32], mybir.dt.bfloat16, kind="ExternalInput")
    b1s = nc.dram_tensor("b1s", [P, 1], mybir.dt.float32, kind="ExternalInput")
    b2s = nc.dram_tensor("b2s", [P, 1], mybir.dt.float32, kind="ExternalInput")
    ident = nc.dram_tensor("ident", [P, P], mybir.dt.bfloat16, kind="ExternalInput")

    # out[chunk, row, col] f32; col = tile*512 + c (tail cols 2048:2560 rows 0:32)
    out = nc.dram_tensor("out", [CHUNKS_PER_CORE, P, 2560],
                         mybir.dt.float32, kind="ExternalOutput")

    with tile.TileContext(nc) as tc:
        with tc.tile_pool(name="const", bufs=1) as cp, \
             tc.tile_pool(name="io", bufs=3) as io, \
             tc.tile_pool(name="iog", bufs=10) as iog, \
             tc.tile_pool(name="mid", bufs=2) as mid, \
             tc.tile_pool(name="psT", bufs=2, space="PSUM") as psT, \
             tc.tile_pool(name="psB", bufs=2, space="PSUM") as psB, \
             tc.tile_pool(name="psC", bufs=2, space="PSUM") as psC:

            t_w0f = cp.tile([P, WID], mybir.dt.bfloat16, tag="w0f")
            t_w1 = cp.tile([P, P], mybir.dt.bfloat16, tag="w1bd")
            t_w2 = cp.tile([P, 32], mybir.dt.bfloat16, tag="w2bd")
            t_b1 = cp.tile([P, 1], mybir.dt.float32, tag="b1s")
            t_b2 = cp.tile([P, 1], mybir.dt.float32, tag="b2s")
            nc.sync.dma_start(out=t_w0f[:], in_=w0f[:])
            nc.sync.dma_start(out=t_w1[:], in_=w1bd[:])
            nc.sync.dma_start(out=t_w2[:], in_=w2bd[:])
            nc.sync.dma_start(out=t_b1[:], in_=b1s[:])
            nc.sync.dma_start(out=t_b2[:], in_=b2s[:])
            t_ident = cp.tile([P, P], mybir.dt.bfloat16, tag="ident")
            nc.sync.dma_start(out=t_ident[:], in_=ident[:])

            for ch in range(CHUNKS_PER_CORE):
                pbase = ch * S_CHUNK          # point base within core
                t_idxall = io.tile([P, IDXC], mybir.dt.int16, tag="idxch")
                nc.sync.dma_start(out=t_idxall[:],
                                  in_=idx16[:, ch * IDXC:(ch + 1) * IDXC])
                t_lg = mid.tile([P, 2560], mybir.dt.float32, tag="lg")

                for ti, T in enumerate(TILES):
                    toff = sum(TILES[:ti])
                    tbase = pbase + toff
                    icol0 = toff // 16

                    # point-major gathers, 2048 idxs per call, one tile per
                    # call so ring usage stays bounded and queues rotate
                    gts = []
                    for sub in range(0, T, GQ):
                        n_sub = min(GQ, T - sub)
                        t_gp = iog.tile([P, n_sub], mybir.dt.bfloat16, tag="gp")
                        nc.gpsimd.dma_gather(
                            out_ap=t_gp[:].rearrange("p (a n) -> p a n",
                                                     a=n_sub // P),
                            in_ap=tbl[ch * ROWS_PER_CHUNK:(ch + 1) * ROWS_PER_CHUNK, :],
                            idxs_ap=t_idxall[:, icol0 + sub // 16:
                                             icol0 + (sub + n_sub) // 16],
                            num_idxs=n_sub,
                            num_idxs_reg=n_sub,
                            elem_size=P,
                            transpose=False,
                            single_packet=False,
                            queue_num=gather_n[0] % 4,
                        )
                        gather_n[0] += 1
                        gts.append(t_gp)

                    # feature tile: rows 0:96 = weighted corners, rows
                    # 96:128 = view-dir embedding + ones row (DMA'd in place)
                    t_gs = mid.tile([P, T], mybir.dt.bfloat16, tag="gs")
                    nc.sync.dma_start(out=t_gs[96:128, :],
                                      in_=vdt[:, tbase:tbase + T])
                    gb = (pbase + toff) // P       # first 128-pt block index
                    t_w8 = io.tile([P, (T // P) * 8], mybir.dt.bfloat16, tag="w8")
                    nc.sync.dma_start(out=t_w8[:],
                                      in_=w8g[:, gb * 8:(gb + T // P) * 8])
                    t_gw = mid.tile([P, (T // P) * 96], mybir.dt.bfloat16,
                                    tag="gw")

                    n_s = T // 512
                    for s in range(n_s):
                        sl = slice(s * 512, (s + 1) * 512)
                        # corner-weight multiply in point-major orientation
                        # (features packed ch-major: f = ch*8 + corner, so the
                        # per-point 8 weights broadcast along ch via stride-0)
                        gpb = GQ // P                  # blocks per gather tile
                        t_src = gts[(s * 4) // gpb]
                        sb = (s * 4) % gpb
                        in0 = (t_src[:].rearrange("p (g f) -> p g f", f=P)
                               [:, sb:sb + 4, 0:96]
                               .rearrange("p g (ch c) -> p g ch c", c=8))
                        in1 = (t_w8[:].rearrange("p (g c) -> p g c", c=8)
                               [:, s * 4:(s + 1) * 4, :]
                               .unsqueeze(2).broadcast_to((P, 4, 12, 8)))
                        outw = (t_gw[:].rearrange("p (g f) -> p g f", f=96)
                                [:, s * 4:(s + 1) * 4, :]
                                .rearrange("p g (ch c) -> p g ch c", c=8))
                        nc.vector.tensor_tensor(out=outw, in0=in0, in1=in1,
                                                op=mybir.AluOpType.mult)
                        # transpose 4 weighted point-major blocks to
                        # feature-major, then plain evacuation on ACT
                        ps_tr = psT.tile([96, 512], mybir.dt.bfloat16, tag="tr")
                        for b in range(4):
                            g = s * 4 + b                 # block within tile
                            nc.tensor.transpose(
                                ps_tr[:, b * P:(b + 1) * P],
                                t_gw[:, g * 96:(g + 1) * 96],
                                t_ident[:])
                        nc.scalar.activation(
                            out=t_gs[0:96, sl], in_=ps_tr[:],
                            func=mybir.ActivationFunctionType.Identity)

                    for pr in range(n_s // 2):
                        s0, s1 = 2 * pr, 2 * pr + 1
                        ps_h1 = psB.tile([P, 512], mybir.dt.float32, tag="psh1")
                        for half, s in enumerate([s0, s1]):
                            sl = slice(s * 512, (s + 1) * 512)
                            nc.tensor.matmul(out=ps_h1[64 * half:64 * half + 64, :],
                                             lhsT=t_w0f[:], rhs=t_gs[:, sl],
                                             start=True, stop=True)
                        t_h1 = mid.tile([P, 512], mybir.dt.bfloat16, tag="h1")
                        nc.vector.tensor_scalar_max(out=t_h1[:], in0=ps_h1[:],
                                                    scalar1=0.0)

                        ps_h2 = psB.tile([P, 512], mybir.dt.float32, tag="psh2")
                        nc.tensor.matmul(out=ps_h2[:], lhsT=t_w1[:], rhs=t_h1[:],
                                         start=True, stop=True)
                        t_h2 = mid.tile([P, 512], mybir.dt.bfloat16, tag="h2")
                        nc.scalar.activation(out=t_h2[:], in_=ps_h2[:],
                                             func=mybir.ActivationFunctionType.Relu,
                                             bias=t_b1[:])

                        if pr % 4 == 0:
                            ps_lg = psC.tile([P, 512], mybir.dt.float32, tag="pslg")
                        ob = 32 * (pr % 4)
                        nc.tensor.matmul(out=ps_lg[ob:ob + 32, :], lhsT=t_w2[:],
                                         rhs=t_h2[:], start=True, stop=True,
                                         tile_position=(0, ob))

                        last_pair = pr == n_s // 2 - 1
                        if (pr % 4 == 3) or last_pair:
                            grows = slice(0, ob + 32)
                            gcol = slice(ti * 512, (ti + 1) * 512)
                            nc.scalar.activation(
                                out=t_lg[grows, gcol], in_=ps_lg[grows, :],
                                func=mybir.ActivationFunctionType.Identity,
                                bias=t_b2[grows, :])

                    if n_s % 2 == 1:
                        # unpaired tail slice: single-width (64-row) MLP
                        s = n_s - 1
                        sl = slice(s * 512, (s + 1) * 512)
                        ps_h1 = psB.tile([P, 512], mybir.dt.float32, tag="psh1")
                        nc.tensor.matmul(out=ps_h1[0:64, :], lhsT=t_w0f[:],
                                         rhs=t_gs[:, sl], start=True, stop=True)
                        t_h1 = mid.tile([P, 512], mybir.dt.bfloat16, tag="h1")
                        nc.vector.tensor_scalar_max(out=t_h1[0:64, :],
                                                    in0=ps_h1[0:64, :],
                                                    scalar1=0.0)
                        ps_h2 = psB.tile([P, 512], mybir.dt.float32, tag="psh2")
                        nc.tensor.matmul(out=ps_h2[0:64, :],
                                         lhsT=t_w1[0:64, 0:64],
                                         rhs=t_h1[0:64, :],
                                         start=True, stop=True)
                        t_h2 = mid.tile([P, 512], mybir.dt.bfloat16, tag="h2")
                        nc.scalar.activation(out=t_h2[0:64, :],
                                             in_=ps_h2[0:64, :],
                                             func=mybir.ActivationFunctionType.Relu,
                                             bias=t_b1[0:64, :])
                        ps_lg = psC.tile([P, 512], mybir.dt.float32, tag="pslg")
                        nc.tensor.matmul(out=ps_lg[0:8, :],
                                         lhsT=t_w2[0:64, 0:8],
                                         rhs=t_h2[0:64, :],
                                         start=True, stop=True,
                                         tile_position=(0, 0))
                        gcol = slice(ti * 512, (ti + 1) * 512)
                        nc.scalar.activation(
                            out=t_lg[0:8, gcol], in_=ps_lg[0:8, :],
                            func=mybir.ActivationFunctionType.Identity,
                            bias=t_b2[0:8, :])

                nc.sync.dma_start(out=out[ch, :, 0:2048], in_=t_lg[:, 0:2048])
                nc.sync.dma_start(out=out[ch, 0:8, 2048:2560],
                                  in_=t_lg[0:8, 2048:2560])

    nc.compile()
    return nc


# ---------------------------------------------------------------------------
# host-side preprocessing
# ---------------------------------------------------------------------------

def _build_tables(k0_grid, w0, b0, w1, b1, w2, b2, viewdirs):
    """Grid-sized + const prep (shared across cores)."""
    g = np.asarray(k0_grid, np.float32)                    # [12,128,128,128]
    gt = np.transpose(g, (1, 2, 3, 0))                     # [x,y,z,12]
    gp = np.pad(gt, [(0, 1), (0, 1), (0, 1), (0, 0)], mode="edge")  # [129,129,129,12]
    nbr = np.zeros((GS, GS, GS, 128), dtype=bf16)
    for ci, (cx, cy, cz) in enumerate([(a, b, c) for a in (0, 1) for b in (0, 1) for c in (0, 1)]):
        # ch-major feature packing: f = ch*8 + corner
        nbr[:, :, :, ci:96:8] = gp[cx:cx + GS, cy:cy + GS, cz:cz + GS, :].astype(bf16)
    nbr = nbr.reshape(N_CHUNKS, ROWS_PER_CHUNK, 128)

    vf = (2.0 ** np.arange(PE_F)).astype(np.float32)
    vd = np.asarray(viewdirs, np.float32)
    vdf = (vd[:, :, None] * vf).reshape(N_RAYS, 3 * PE_F)
    vd_emb = np.concatenate([vd, np.sin(vdf), np.cos(vdf)], axis=-1)  # [N,27]
    vd_ext = np.zeros((N_RAYS, 32), np.float32)
    vd_ext[:, :27] = vd_emb
    vd_ext[:, 27] = 1.0

    w0 = np.asarray(w0, np.float32); w1 = np.asarray(w1, np.float32)
    w2 = np.asarray(w2, np.float32)
    b0 = np.asarray(b0, np.float32); b1 = np.asarray(b1, np.float32)
    b2 = np.asarray(b2, np.float32)

    w0f = np.zeros((128, WID), np.float32)
    for r in range(96):
        w0f[r] = w0[r // 8]          # ch-major: row r = (ch = r//8, corner r%8)
    w0f[96:123] = w0[12:39]
    w0f[123] = b0
    w1bd = np.zeros((128, 128), np.float32)
    w1bd[0:64, 0:64] = w1
    w1bd[64:128, 64:128] = w1
    w2bd = np.zeros((128, 32), np.float32)
    w2bd[0:64, 0:8] = w2
    w2bd[64:128, 8:16] = w2
    b1v = np.concatenate([b1, b1]).reshape(128, 1).astype(np.float32)
    b2v = np.zeros((128, 1), np.float32)
    for gidx in range(4):
        b2v[32 * gidx:32 * gidx + 8, 0] = b2
        b2v[32 * gidx + 8:32 * gidx + 16, 0] = b2
    consts = dict(
        w0f=w0f.astype(bf16), w1bd=w1bd.astype(bf16),
        w2bd=w2bd.astype(bf16), b1s=b1v, b2s=b2v,
        ident=np.eye(128, dtype=bf16))
    return nbr, vd_ext, consts


def kernel(ray_pts, viewdirs, k0_grid, w0, b0, w1, b1, w2, b2, ray_id):
    from concourse import bass_utils

    pts = np.asarray(ray_pts, np.float32)
    rid = np.asarray(ray_id, np.int64).astype(np.int32)
    m = pts.shape[0]
    assert m == M, f"kernel compiled for M={M}, got {m}"

    nbr, vd_ext, consts = _build_tables(k0_grid, w0, b0, w1, b1, w2, b2, viewdirs)

    # ---- per-point host math ----
    pix = pts * np.float32(GS - 1)
    lo = np.clip(np.floor(pix), 0, GS - 1).astype(np.int32)     # [M,3]
    frac = pix - lo.astype(np.float32)                           # [M,3] in [0,1]
    chunk_id = (lo[:, 0] >> 1).astype(np.int32)                  # [M] in [0,64)
    loc_idx = ((lo[:, 0] & 1) << 14) | (lo[:, 1] << 7) | lo[:, 2]  # in-chunk row

    fx, fy, fz = frac[:, 0], frac[:, 1], frac[:, 2]
    wx = np.stack([1.0 - fx, fx], 1)
    wy = np.stack([1.0 - fy, fy], 1)
    wz = np.stack([1.0 - fz, fz], 1)
    w8 = (wx[:, :, None, None] * wy[:, None, :, None] * wz[:, None, None, :]
          ).reshape(m, 8)                                        # corner c = cx*4+cy*2+cz

    # ---- sort by chunk, pad to S_CHUNK ----
    cnt = np.bincount(chunk_id, minlength=N_CHUNKS)
    order = np.argsort(chunk_id, kind="stable")
    overflow_slots = None
    if cnt.max() > S_CHUNK:
        # pathological distribution: process overflow points on host
        keep = np.ones(m, bool)
        cum = np.zeros(N_CHUNKS + 1, np.int64)
        np.cumsum(cnt, out=cum[1:])
        for c in np.where(cnt > S_CHUNK)[0]:
            drop = order[cum[c] + S_CHUNK:cum[c + 1]]
            keep[drop] = False
        overflow_slots = np.where(~keep)[0]
        chunk_id2 = chunk_id[keep]
        order = np.argsort(chunk_id2, kind="stable")
        # re-map below uses masked arrays; simplest: recompute on the kept subset
        idx_kept = np.where(keep)[0]
        order = idx_kept[order]
        cnt = np.minimum(cnt, S_CHUNK)

    cum = np.zeros(N_CHUNKS + 1, np.int64)
    np.cumsum(cnt, out=cum[1:])
    rank = np.arange(len(order)) - np.repeat(cum[:-1], cnt)
    slots = (np.repeat(np.arange(N_CHUNKS), cnt) * S_CHUNK + rank).astype(np.int64)
    slot_src = np.full(N_CHUNKS * S_CHUNK, -1, np.int64)
    slot_src[slots] = order

    valid = slot_src >= 0
    src = np.where(valid, slot_src, 0)

    loc_s = np.where(valid, loc_idx[src], 0).astype(np.int16)    # [64*S]
    w8pm = np.where(valid[:, None], w8[src], 0.0).astype(bf16)   # [64*S, 8]
    vd_s = np.where(valid[:, None], vd_ext[rid[src]], 0.0).astype(bf16).T.copy()  # [32, 64*S]

    # idx16 per (chunk, tile): [128, T/16] replicated
    idx_cols = np.empty((128, N_CHUNKS * IDXC), np.int16)
    for c in range(N_CHUNKS):
        base = c * S_CHUNK
        off = 0
        for T in TILES:
            v = loc_s[base + off: base + off + T].reshape(T // 16, 16).T  # [16, T/16]
            colb = c * IDXC + off // 16
            idx_cols[:, colb:colb + T // 16] = np.tile(v, (8, 1))
            off += T

    # ---- build per-core input maps ----
    key = "prog"
    if key not in _cache:
        _cache[key] = _build_program()
    nc = _cache[key]
    kernel.last_nc = nc

    in_maps = []
    for k in range(N_CORES):
        c0, c1 = k * CHUNKS_PER_CORE, (k + 1) * CHUNKS_PER_CORE
        p0, p1 = c0 * S_CHUNK, c1 * S_CHUNK
        im = dict(consts)
        im["tbl"] = np.ascontiguousarray(nbr[c0:c1].reshape(-1, 128))
        im["idx16"] = np.ascontiguousarray(idx_cols[:, c0 * IDXC:c1 * IDXC])
        # per-point corner weights grouped by 128-pt gather block:
        # w8g[p, g*8 + c] = weight of slot g*128+p, corner c
        im["w8g"] = np.ascontiguousarray(
            w8pm[p0:p1].reshape(-1, 128, 8).transpose(1, 0, 2).reshape(128, -1))
        im["vdt"] = np.ascontiguousarray(vd_s[:, p0:p1])
        in_maps.append(im)

    kernel.last_in_maps = in_maps
    sim_cores = os.environ.get("KERNEL_SIM_CORES", "")
    if sim_cores:
        from concourse.bass_interp import CoreSim
        outs = [np.zeros((CHUNKS_PER_CORE, 128, 2560), np.float32)
                for _ in range(N_CORES)]
        want_trace = os.environ.get("KERNEL_SIM_TRACE", "0") == "1"
        for k in [int(x) for x in sim_cores.split(",")]:
            sim = CoreSim(nc, trace=want_trace, publish_trace=want_trace)
            for name, arr in in_maps[k].items():
                sim.tensor(name)[:] = arr
            sim.simulate()
            print(f"[core {k}] CoreSim modeled time: {sim.time} ns")
            kernel.last_sim_time = sim.time
            outs[k] = np.asarray(sim.mem_tensor("out")).reshape(
                CHUNKS_PER_CORE, 128, 2560).copy()
        kernel.last_results = None
    else:
        trace = os.environ.get("KERNEL_TRACE", "0") == "1"
        res = bass_utils.run_bass_kernel_spmd(
            nc, in_maps, core_ids=list(range(N_CORES)), trace=trace)
        kernel.last_results = res
        outs = [r["out"] for r in res.results]     # each [8, 128, 2560] f32

    # ---- unshard: out[chunk, row, tile*512+col] -> logits per slot ----
    allout = np.stack(outs)                        # [cores, 8, 128, 2560]
    allout = allout.reshape(N_CHUNKS, 128, 2560)

    si = np.arange(N_CHUNKS * S_CHUNK)
    sc = si % S_CHUNK
    chf = si // S_CHUNK
    tile_idx = np.minimum(sc // 4096, len(TILES) - 1)
    within = sc - tile_idx * 4096
    s_id = within // 512
    col = tile_idx * 512 + within % 512
    pr = s_id // 2
    half = s_id % 2
    row = 32 * (pr % 4) + 8 * half

    logits_slots = allout[chf[:, None], row[:, None] + np.arange(8)[None, :], col[:, None]]

    logits = np.empty((m, E), np.float32)
    logits[slot_src[valid]] = logits_slots[valid]

    if overflow_slots is not None and len(overflow_slots):
        logits[overflow_slots] = _host_reference_points(
            pts[overflow_slots], rid[overflow_slots], np.asarray(k0_grid, np.float32),
            w0, b0, w1, b1, w2, b2, vd_ext)
    return logits


def _host_reference_points(pts, rid, grid, w0, b0, w1, b1, w2, b2, vd_ext):
    pix = pts * np.float32(GS - 1)
    lo = np.clip(np.floor(pix), 0, GS - 1).astype(np.int32)
    hi = np.clip(lo + 1, 0, GS - 1)
    f = pix - lo
    acc = 0.0
    for ci, (cx, cy, cz) in enumerate([(a, b, c) for a in (0, 1) for b in (0, 1) for c in (0, 1)]):
        ix = np.where(cx, hi[:, 0], lo[:, 0]); iy = np.where(cy, hi[:, 1], lo[:, 1])
        iz = np.where(cz, hi[:, 2], lo[:, 2])
        wgt = (np.where(cx, f[:, 0], 1 - f[:, 0]) * np.where(cy, f[:, 1], 1 - f[:, 1])
               * np.where(cz, f[:, 2], 1 - f[:, 2]))
        acc = acc + grid[:, ix, iy, iz].T * wgt[:, None]
    feat = np.concatenate([acc, vd_ext[rid][:, :27]], -1)
    h = np.maximum(feat @ np.asarray(w0) + np.asarray(b0), 0)
    h = np.maximum(h @ np.asarray(w1) + np.asarray(b1), 0)
    return (h @ np.asarray(w2) + np.asarray(b2)).astype(np.float32)

